# revision 28
# baseline (speedup 1.0000x reference)
"""Trainium2 Bass kernel for nn_Attention_21835613733572 — v9 (j8, P2, fp8 k, deep v + bf16 tree).

reference:
    score = einsum('bci,bcj->bij', k, q) / sqrt(L)       # (B, L, L)
    score = softmax(score, axis=0)                       # over the BATCH axis
    out   = einsum('bci,bij->bcj', v, score)             # (B, C, L)
with B, C, L = 16, 512, 1024 (f32 inputs/outputs).

Distribution: 8 j-slices of 128 columns, one per core; every core holds the
full k and v (the batch-axis softmax needs all 16 batches per (i,j), and
collectives on this fleet cost a fixed ~85us per NEFF — measured — so any
cross-core exchange loses). This is the baseline's sharding, rebuilt around
what the baseline trace showed:

 * P2 layout: MM1 uses stationary k[c,i_tile], moving q[c,j], so scores land
   as e[i_part, j_free] — exactly the lhsT layout MM2 needs. Removes all PE
   transpose matmuls (~14us of PE) the baseline spent.
 * ScalarE runs ONLY Exp (one activation table load, 1.28us per swap
   otherwise): MM1 psums are batch-quad packed ([128,4,128] = one full psum
   bank) so exp is 32 instructions instead of 128+ mixed exp/copy.
 * Denominator d[i,j] = sum_b e[b,i,j] via a 4-level strided pairwise fold
   on VectorE (4 instructions per i-tile, operands [128,8,128]...), not a
   15-add scalar tree.
 * MM2 psum evacuation on VectorE (no scalar table thrash), output DMA on
   the ACT hardware queue (the baseline put it on gpsimd's software DGE,
   which sustained only ~50 GB/s and added a ~30us tail).
 * k then v stream on the sync hardware queue (baseline showed a single
   HWDGE queue sustains ~385 GB/s; total per-core HBM ~420 GB/s).

Per-core input DMA 27.3MB (k fp8 8.4 + q 2.1 + v 16.8). Measured on this
fleet: ~98.4us HW exec, rel err 1.777e-2 (gate 2e-2; deterministic for the
fixed-seed harness inputs). The kernel is bound by DMA-during-compute:
~300-320 GB/s effective while PE/ACT stream SBUF operands (vs ~420 free),
so the last v tile lands ~t=91us, plus ~9us fixed NEFF startup and ~6us
exit drain. Experiments that regressed or tied (kernel_v5..v13): splitting
k or v across both HWDGE queues, SBUF-staging the exp input to dodge the
ACT psum-access penalty, deeper/shallower pool rings, tail-split d-trees,
half-tile tail v DMAs. Collectives (incl. chunked 2-rank AllGathers) cost
a fixed ~85us/NEFF here, ruling out batch-sharded softmax designs.
"""
import sys

sys.path.insert(0, "/opt/trn_rl_repo")

import numpy as np
import ml_dtypes

try:
    import types
    import antenv
    import trn_agent_boot.trn_boot as _tb
    if "antenv.axon_hooks" not in sys.modules:
        _hook = _tb._ntff_profile_via_ctypes("/opt/axon/libaxon_pjrt.so")
        _mod = types.ModuleType("antenv.axon_hooks")
        _mod.get_axon_ntff_profile_hook = lambda: _hook
        _mod.set_axon_ntff_profile_hook = lambda h: None
        sys.modules["antenv.axon_hooks"] = _mod
        antenv.axon_hooks = _mod
except Exception:
    pass

import concourse.bass as bass
import concourse.mybir as mybir
import concourse.tile as tile
from concourse import bacc
from concourse.bass_utils import run_bass_kernel_spmd

P = 128
B, C, L = 16, 512, 1024
NCORES = 8
J = L // NCORES   # 128 j-columns per core
BF16 = mybir.dt.bfloat16
F32 = mybir.dt.float32
FP8 = mybir.dt.float8e4
CC_N = C // P     # 4 contraction tiles
IT_N = L // P     # 8 i-tiles
BQ_N = B // 4     # 4 batch-quads

_cached_nc = None


def _build():
    nc = bacc.Bacc("TRN2", target_bir_lowering=False, debug=False,
                   num_devices=NCORES)
    q_ext = nc.dram_tensor("q", [P, B, CC_N, J], BF16, kind="ExternalInput").ap()
    k_ext = nc.dram_tensor("k", [P, IT_N, B, CC_N, P], FP8,
                           kind="ExternalInput").ap()
    v_ext = nc.dram_tensor("v", [P, B, IT_N, C], BF16,
                           kind="ExternalInput").ap()
    out_ext = nc.dram_tensor("out", [P, B, C], BF16,
                             kind="ExternalOutput").ap()

    with tile.TileContext(nc) as tc:
        with (
            tc.tile_pool(name="qpool", bufs=1) as qpool,
            tc.tile_pool(name="kpool", bufs=5) as kpool,
            tc.tile_pool(name="vpool", bufs=12) as vpool,
            tc.tile_pool(name="epool", bufs=1) as epool,
            tc.tile_pool(name="lpool", bufs=2) as lpool,
            tc.tile_pool(name="dpool", bufs=4) as dpool,
            tc.tile_pool(name="spool", bufs=1) as spool,
            tc.tile_pool(name="opool", bufs=2) as opool,
            tc.tile_pool(name="ps1", bufs=3, space="PSUM") as ps1,
            tc.tile_pool(name="ps2", bufs=4, space="PSUM") as ps2,
        ):
            # q on the ACT queue (done ~5us); k then v own the sync queue.
            q_sb = qpool.tile([P, B, CC_N, J], BF16, name="q_all")
            for qs in range(4):
                nc.scalar.dma_start(q_sb[:, qs * 4:(qs + 1) * 4],
                                    q_ext[:, qs * 4:(qs + 1) * 4])

            k_sb = []
            for it in range(IT_N):
                kt = kpool.tile([P, B, CC_N, P], FP8, tag="ktile")
                if it == 0:
                    nc.sync.dma_start(kt[:, 0:4], k_ext[:, it, 0:4])
                    nc.sync.dma_start(kt[:, 4:], k_ext[:, it, 4:])
                else:
                    nc.sync.dma_start(kt[:], k_ext[:, it])
                k_sb.append(kt)
            v_sb = []
            for b in range(B):
                vt = vpool.tile([P, IT_N, C], BF16, tag="vtile")
                nc.sync.dma_start(vt[:], v_ext[:, b])
                v_sb.append(vt)

            # e[i_part, it, b, j]: all exp'd scores, bf16, 32KB/partition
            e_st = epool.tile([P, IT_N, B, J], BF16, name="e_st")
            recip_bf = spool.tile([P, IT_N, J], BF16, name="recip_bf")

            # ---- MM1 + exp + d + recip, i-tile major ----
            for it in range(IT_N):
                kt = k_sb[it]
                for bq in range(BQ_N):
                    ps = ps1.tile([P, 4, J], F32, tag="mm1")
                    for s in range(4):
                        b = bq * 4 + s
                        for cc in range(CC_N):
                            nc.tensor.matmul(
                                ps[:, s],
                                kt[:, b, cc, :],
                                q_sb[:, b, cc, :],
                                start=(cc == 0),
                                stop=(cc == CC_N - 1),
                            )
                    nc.scalar.activation(
                        e_st[:, it, bq * 4:(bq + 1) * 4, :], ps[:],
                        mybir.ActivationFunctionType.Exp,
                        scale=float(1.0 / (L ** 0.5)),
                    )
                # d[it] via strided pairwise folds: 16 -> 8 -> 4 -> 2 -> 1
                l8 = lpool.tile([P, 8, J], BF16, tag="l8")
                nc.vector.tensor_add(
                    l8[:], e_st[:, it, 0:16:2, :], e_st[:, it, 1:16:2, :])
                l4 = lpool.tile([P, 4, J], BF16, tag="l4")
                nc.vector.tensor_add(l4[:], l8[:, 0:8:2, :], l8[:, 1:8:2, :])
                l2 = lpool.tile([P, 2, J], BF16, tag="l2")
                nc.vector.tensor_add(l2[:], l4[:, 0:4:2, :], l4[:, 1:4:2, :])
                d32 = dpool.tile([P, J], F32, tag="d32")
                nc.vector.tensor_add(d32[:], l2[:, 0, :], l2[:, 1, :])
                r32 = dpool.tile([P, J], F32, tag="r32")
                nc.vector.reciprocal_approx_fast(r32[:], d32[:])
                nc.vector.tensor_copy(recip_bf[:, it, :], r32[:])
                if it == 3:
                    # first-half probs: runs on VectorE during MM1 of it 4-7
                    for b in range(B):
                        nc.vector.tensor_mul(
                            e_st[:, 0:4, b, :], e_st[:, 0:4, b, :],
                            recip_bf[:, 0:4, :])

            # ---- probs (second half) up front, then MM2 ----
            # Hoisting all prob muls lets MM2 chains run back-to-back on PE
            # instead of waiting cast(b-1) -> prob(b) on VectorE each batch.
            for b in range(B):
                nc.vector.tensor_mul(
                    e_st[:, 4:8, b, :], e_st[:, 4:8, b, :], recip_bf[:, 4:8, :])
            # out DMAs merged 4 batches at a time: out completions recycle
            # into the shared DMA-semaphore pool that also gates the late v
            # dma_start issues on the sync engine; 16 late-completing out
            # DMAs there stall the v stream against MM2 (measured ~5us).
            ot = None
            for b in range(B):
                vt = v_sb[b]
                po = ps2.tile([P, C], F32, tag="mm2")
                for it in range(IT_N):
                    nc.tensor.matmul(
                        po[:],
                        e_st[:, it, b, :],
                        vt[:, it, :],
                        start=(it == 0),
                        stop=(it == IT_N - 1),
                    )
                if b % 4 == 0:
                    ot = opool.tile([P, 4, C], BF16, tag="otile")
                nc.vector.tensor_copy(ot[:, b % 4], po[:])
                if b % 4 == 3:
                    # outs ride the sync queue: scheduled after all v
                    # issues, so their late completions recycle into sems
                    # behind the v tail instead of ahead of it. Q1 is free
                    # of v by the time out bytes are ready.
                    nc.sync.dma_start(out_ext[:, b - 3:b + 1], ot[:])

    nc.compile()
    return nc


def _prep_inputs(q, k, v):
    """Host-side bf16 cast + DMA-friendly layouts. k_p/v_p shared by cores."""
    q_bf = np.asarray(q).astype(ml_dtypes.bfloat16)
    k_f8 = np.asarray(k).astype(ml_dtypes.float8_e4m3fn)
    v_bf = np.asarray(v).astype(ml_dtypes.bfloat16)

    # k: (B, C, L) -> (c_in 128, it 8, b 16, cc 4, i_in 128)
    k_p = np.ascontiguousarray(
        k_f8.reshape(B, CC_N, P, IT_N, P).transpose(2, 3, 0, 1, 4))
    # v: (B, C, L) -> (i_in 128, b 16, it 8, c 512)
    v_p = np.ascontiguousarray(
        v_bf.reshape(B, C, IT_N, P).transpose(3, 0, 2, 1))
    # q per j-slice: (c_in 128, b 16, cc 4, j 128)
    in_maps = []
    for js in range(NCORES):
        qs = q_bf[:, :, js * J:(js + 1) * J]
        q_p = np.ascontiguousarray(
            qs.reshape(B, CC_N, P, J).transpose(2, 0, 1, 3))
        in_maps.append({"q": q_p, "k": k_p, "v": v_p})
    return in_maps


def kernel(q: np.ndarray, k: np.ndarray, v: np.ndarray) -> np.ndarray:
    """Full inputs (B, C, L) f32 -> full output (B, C, L) f32."""
    global _cached_nc
    assert q.shape == (B, C, L) and k.shape == (B, C, L) and v.shape == (B, C, L)

    in_maps = _prep_inputs(q, k, v)
    if _cached_nc is None:
        _cached_nc = _build()
    res = run_bass_kernel_spmd(_cached_nc, in_maps, list(range(NCORES)))

    # out param: (j_in 128, b 16, c 512); out[b, c, js*128+j_in] = arr[j_in, b, c]
    out = np.concatenate(
        [np.asarray(res.results[core]["out"]).astype(np.float32)
         .transpose(1, 2, 0) for core in range(NCORES)], axis=2)
    return np.ascontiguousarray(out)


if __name__ == "__main__":
    rng = np.random.default_rng(0)
    q = rng.standard_normal((B, C, L)).astype(np.float32)
    k = rng.standard_normal((B, C, L)).astype(np.float32)
    v = rng.standard_normal((B, C, L)).astype(np.float32)
    out = kernel(q=q, k=k, v=v)
    s = np.einsum("bci,bcj->bij", k, q) / np.sqrt(L)
    e = np.exp(s - s.max(axis=0, keepdims=True))
    p = e / e.sum(axis=0, keepdims=True)
    ref = np.einsum("bci,bij->bcj", v, p)
    print("rel fro err:", np.linalg.norm(out - ref) / np.linalg.norm(ref))



# revision 31
# speedup vs baseline: 1.1579x; 1.1579x over previous
"""Trainium2 Bass kernel for nn_Attention_21835613733572 — v9 (j8, P2, fp8 k, deep v + bf16 tree).

reference:
    score = einsum('bci,bcj->bij', k, q) / sqrt(L)       # (B, L, L)
    score = softmax(score, axis=0)                       # over the BATCH axis
    out   = einsum('bci,bij->bcj', v, score)             # (B, C, L)
with B, C, L = 16, 512, 1024 (f32 inputs/outputs).

Distribution: 8 j-slices of 128 columns, one per core; every core holds the
full k and v (the batch-axis softmax needs all 16 batches per (i,j), and
collectives on this fleet cost a fixed ~85us per NEFF — measured — so any
cross-core exchange loses). This is the baseline's sharding, rebuilt around
what the baseline trace showed:

 * P2 layout: MM1 uses stationary k[c,i_tile], moving q[c,j], so scores land
   as e[i_part, j_free] — exactly the lhsT layout MM2 needs. Removes all PE
   transpose matmuls (~14us of PE) the baseline spent.
 * ScalarE runs ONLY Exp (one activation table load, 1.28us per swap
   otherwise): MM1 psums are batch-quad packed ([128,4,128] = one full psum
   bank) so exp is 32 instructions instead of 128+ mixed exp/copy.
 * Denominator d[i,j] = sum_b e[b,i,j] via a 4-level strided pairwise fold
   on VectorE (4 instructions per i-tile, operands [128,8,128]...), not a
   15-add scalar tree.
 * MM2 psum evacuation on VectorE (no scalar table thrash), output DMA on
   the ACT hardware queue (the baseline put it on gpsimd's software DGE,
   which sustained only ~50 GB/s and added a ~30us tail).
 * k then v stream on the sync hardware queue (baseline showed a single
   HWDGE queue sustains ~385 GB/s; total per-core HBM ~420 GB/s).

Per-core input DMA 27.3MB (k fp8 8.4 + q 2.1 + v 16.8). Measured on this
fleet: ~98.4us HW exec, rel err 1.777e-2 (gate 2e-2; deterministic for the
fixed-seed harness inputs). The kernel is bound by DMA-during-compute:
~300-320 GB/s effective while PE/ACT stream SBUF operands (vs ~420 free),
so the last v tile lands ~t=91us, plus ~9us fixed NEFF startup and ~6us
exit drain. Experiments that regressed or tied (kernel_v5..v13): splitting
k or v across both HWDGE queues, SBUF-staging the exp input to dodge the
ACT psum-access penalty, deeper/shallower pool rings, tail-split d-trees,
half-tile tail v DMAs. Collectives (incl. chunked 2-rank AllGathers) cost
a fixed ~85us/NEFF here, ruling out batch-sharded softmax designs.
"""
import sys

sys.path.insert(0, "/opt/trn_rl_repo")

import numpy as np
import ml_dtypes

try:
    import types
    import antenv
    import trn_agent_boot.trn_boot as _tb
    if "antenv.axon_hooks" not in sys.modules:
        _hook = _tb._ntff_profile_via_ctypes("/opt/axon/libaxon_pjrt.so")
        _mod = types.ModuleType("antenv.axon_hooks")
        _mod.get_axon_ntff_profile_hook = lambda: _hook
        _mod.set_axon_ntff_profile_hook = lambda h: None
        sys.modules["antenv.axon_hooks"] = _mod
        antenv.axon_hooks = _mod
except Exception:
    pass

import concourse.bass as bass
import concourse.mybir as mybir
import concourse.tile as tile
from concourse import bacc
from concourse.bass_utils import run_bass_kernel_spmd

P = 128
B, C, L = 16, 512, 1024
NCORES = 8
J = L // NCORES   # 128 j-columns per core
BF16 = mybir.dt.bfloat16
F32 = mybir.dt.float32
FP8 = mybir.dt.float8e4
CC_N = C // P     # 4 contraction tiles
IT_N = L // P     # 8 i-tiles
BQ_N = B // 4     # 4 batch-quads

_cached_nc = None


def _build():
    nc = bacc.Bacc("TRN2", target_bir_lowering=False, debug=False,
                   num_devices=NCORES)
    q_ext = nc.dram_tensor("q", [P, B, CC_N, J], BF16, kind="ExternalInput").ap()
    k_ext = nc.dram_tensor("k", [P, IT_N, B, CC_N, P], FP8,
                           kind="ExternalInput").ap()
    v_ext = nc.dram_tensor("v", [P, B, IT_N, C], BF16,
                           kind="ExternalInput").ap()
    out_ext = nc.dram_tensor("out", [P, B, C], BF16,
                             kind="ExternalOutput").ap()

    with tile.TileContext(nc) as tc:
        with (
            tc.tile_pool(name="qpool", bufs=1) as qpool,
            tc.tile_pool(name="kpool", bufs=5) as kpool,
            tc.tile_pool(name="vpool", bufs=13) as vpool,
            tc.tile_pool(name="epool", bufs=1) as epool,
            tc.tile_pool(name="lpool", bufs=1) as lpool,
            tc.tile_pool(name="dpool", bufs=2) as dpool,
            tc.tile_pool(name="spool", bufs=1) as spool,
            tc.tile_pool(name="opool", bufs=2) as opool,
            tc.tile_pool(name="ps1", bufs=4, space="PSUM") as ps1,
            tc.tile_pool(name="ps2", bufs=4, space="PSUM") as ps2,
        ):
            # q on the ACT queue (done ~5us); k then v own the sync queue.
            q_sb = qpool.tile([P, B, CC_N, J], BF16, name="q_all")
            for qs in range(4):
                nc.scalar.dma_start(q_sb[:, qs * 4:(qs + 1) * 4],
                                    q_ext[:, qs * 4:(qs + 1) * 4])

            k_sb = []
            for it in range(IT_N):
                kt = kpool.tile([P, B, CC_N, P], FP8, tag="ktile")
                if it == 0:
                    nc.sync.dma_start(kt[:, 0:4], k_ext[:, it, 0:4])
                    nc.sync.dma_start(kt[:, 4:], k_ext[:, it, 4:])
                else:
                    nc.sync.dma_start(kt[:], k_ext[:, it])
                k_sb.append(kt)
            v_sb = []
            for b in range(B):
                vt = vpool.tile([P, IT_N, C], BF16, tag="vtile")
                nc.sync.dma_start(vt[:], v_ext[:, b])
                v_sb.append(vt)

            # e[i_part, it, b, j]: all exp'd scores, bf16, 32KB/partition
            e_st = epool.tile([P, IT_N, B, J], BF16, name="e_st")
            recip_bf = spool.tile([P, IT_N, J], BF16, name="recip_bf")

            # ---- MM1 + exp + d + recip, i-tile major ----
            for it in range(IT_N):
                kt = k_sb[it]
                for bq in range(BQ_N):
                    ps = ps1.tile([P, 4, J], F32, tag="mm1")
                    for s in range(4):
                        b = bq * 4 + s
                        for cc in range(CC_N):
                            nc.tensor.matmul(
                                ps[:, s],
                                kt[:, b, cc, :],
                                q_sb[:, b, cc, :],
                                start=(cc == 0),
                                stop=(cc == CC_N - 1),
                            )
                    nc.scalar.activation(
                        e_st[:, it, bq * 4:(bq + 1) * 4, :], ps[:],
                        mybir.ActivationFunctionType.Exp,
                        scale=float(1.0 / (L ** 0.5)),
                    )
                # d[it] via strided pairwise folds: 16 -> 8 -> 4 -> 2 -> 1
                l8 = lpool.tile([P, 8, J], BF16, tag="l8")
                nc.vector.tensor_add(
                    l8[:], e_st[:, it, 0:16:2, :], e_st[:, it, 1:16:2, :])
                l4 = lpool.tile([P, 4, J], BF16, tag="l4")
                nc.vector.tensor_add(l4[:], l8[:, 0:8:2, :], l8[:, 1:8:2, :])
                l2 = lpool.tile([P, 2, J], BF16, tag="l2")
                nc.vector.tensor_add(l2[:], l4[:, 0:4:2, :], l4[:, 1:4:2, :])
                d32 = dpool.tile([P, J], F32, tag="d32")
                nc.vector.tensor_add(d32[:], l2[:, 0, :], l2[:, 1, :])
                r32 = dpool.tile([P, J], F32, tag="r32")
                nc.vector.reciprocal_approx_fast(r32[:], d32[:])
                nc.vector.tensor_copy(recip_bf[:, it, :], r32[:])
                if it == 3:
                    # first-half probs: runs on VectorE during MM1 of it 4-7
                    for b in range(B):
                        nc.vector.tensor_mul(
                            e_st[:, 0:4, b, :], e_st[:, 0:4, b, :],
                            recip_bf[:, 0:4, :])

            # ---- probs (second half) up front, then MM2 ----
            # Hoisting all prob muls lets MM2 chains run back-to-back on PE
            # instead of waiting cast(b-1) -> prob(b) on VectorE each batch.
            for b in range(B):
                nc.vector.tensor_mul(
                    e_st[:, 4:8, b, :], e_st[:, 4:8, b, :], recip_bf[:, 4:8, :])
            # out DMAs merged 4 batches at a time: out completions recycle
            # into the shared DMA-semaphore pool that also gates the late v
            # dma_start issues on the sync engine; 16 late-completing out
            # DMAs there stall the v stream against MM2 (measured ~5us).
            ot = None
            for b in range(B):
                vt = v_sb[b]
                po = ps2.tile([P, C], F32, tag="mm2")
                for it in range(IT_N):
                    nc.tensor.matmul(
                        po[:],
                        e_st[:, it, b, :],
                        vt[:, it, :],
                        start=(it == 0),
                        stop=(it == IT_N - 1),
                    )
                if b % 4 == 0:
                    ot = opool.tile([P, 4, C], BF16, tag="otile")
                nc.vector.tensor_copy(ot[:, b % 4], po[:])
                if b % 4 == 3:
                    nc.scalar.dma_start(out_ext[:, b - 3:b + 1], ot[:])

    nc.compile()
    return nc


def _prep_inputs(q, k, v):
    """Host-side bf16 cast + DMA-friendly layouts. k_p/v_p shared by cores."""
    q_bf = np.asarray(q).astype(ml_dtypes.bfloat16)
    k_f8 = np.asarray(k).astype(ml_dtypes.float8_e4m3fn)
    v_bf = np.asarray(v).astype(ml_dtypes.bfloat16)

    # k: (B, C, L) -> (c_in 128, it 8, b 16, cc 4, i_in 128)
    k_p = np.ascontiguousarray(
        k_f8.reshape(B, CC_N, P, IT_N, P).transpose(2, 3, 0, 1, 4))
    # v: (B, C, L) -> (i_in 128, b 16, it 8, c 512)
    v_p = np.ascontiguousarray(
        v_bf.reshape(B, C, IT_N, P).transpose(3, 0, 2, 1))
    # q per j-slice: (c_in 128, b 16, cc 4, j 128)
    in_maps = []
    for js in range(NCORES):
        qs = q_bf[:, :, js * J:(js + 1) * J]
        q_p = np.ascontiguousarray(
            qs.reshape(B, CC_N, P, J).transpose(2, 0, 1, 3))
        in_maps.append({"q": q_p, "k": k_p, "v": v_p})
    return in_maps


def kernel(q: np.ndarray, k: np.ndarray, v: np.ndarray) -> np.ndarray:
    """Full inputs (B, C, L) f32 -> full output (B, C, L) f32."""
    global _cached_nc
    assert q.shape == (B, C, L) and k.shape == (B, C, L) and v.shape == (B, C, L)

    in_maps = _prep_inputs(q, k, v)
    if _cached_nc is None:
        _cached_nc = _build()
    res = run_bass_kernel_spmd(_cached_nc, in_maps, list(range(NCORES)))

    # out param: (j_in 128, b 16, c 512); out[b, c, js*128+j_in] = arr[j_in, b, c]
    out = np.concatenate(
        [np.asarray(res.results[core]["out"]).astype(np.float32)
         .transpose(1, 2, 0) for core in range(NCORES)], axis=2)
    return np.ascontiguousarray(out)


if __name__ == "__main__":
    rng = np.random.default_rng(0)
    q = rng.standard_normal((B, C, L)).astype(np.float32)
    k = rng.standard_normal((B, C, L)).astype(np.float32)
    v = rng.standard_normal((B, C, L)).astype(np.float32)
    out = kernel(q=q, k=k, v=v)
    s = np.einsum("bci,bcj->bij", k, q) / np.sqrt(L)
    e = np.exp(s - s.max(axis=0, keepdims=True))
    p = e / e.sum(axis=0, keepdims=True)
    ref = np.einsum("bci,bij->bcj", v, p)
    print("rel fro err:", np.linalg.norm(out - ref) / np.linalg.norm(ref))

